# revision 1
# baseline (speedup 1.0000x reference)
"""AlternatingLSTM Trainium2 kernel (v2).

B=32, T=512, D=H=512, L=8 alternating-direction LSTM stack.
Data-parallel over batch: 4 seqs/core on 8 cores, weights replicated.

Per core, per layer:
  Phase A: xg = x @ Wx.T as bf16 matmuls (N=512), bias folded into the
           PSUM->SBUF copy via per-partition activation bias. Gate-packed
           layout xg[p, t*96 + gate*16 + hc*4 + b].
  Phase B: T_eff sequential steps (T_eff = max(lengths)); per step
           80 small matmuls (N=4) + 9 DVE + 4 ACT ops. No masking:
           padded positions get xg_i/f = -1e9, xg_r = +1e9 so the cell
           emits exact zeros. h state ping-pongs between two fixed
           SBUF buffers (parity by unroll position); the step loop is
           unrolled x8 inside tc.For_i to amortize loop barriers.
Output: final h-slot buffer DMA'd to DRAM in bf16, host transposes.
"""

import os
import numpy as np
import ml_dtypes

B, T, D, H, L = 32, 512, 512, 512, 8
NCORES = 8
BL = B // NCORES          # 4 sequences per core
NL = int(os.environ.get("KERNEL_NLAYERS", str(L)))
UNROLL = int(os.environ.get("KERNEL_UNROLL", "16"))
NDUMMY = int(os.environ.get("KERNEL_NDUMMY", "0"))
STAGGER = bool(int(os.environ.get("KERNEL_STAGGER", "0")))

HS_W = (T + 2) * 16       # h-slot buffer: slot s = t+1
XG_W = T * 96             # 6 gates * 16 per step, t-major


def _prep_host(x, lengths, Wx, Wh, bh):
    """Build per-core input maps (numpy only)."""
    bf16 = ml_dtypes.bfloat16
    # lhsT tile layout: col = (m*4 + k)*128 + c ; value = W[l, m*128+c, k*128+p]
    wxt = np.ascontiguousarray(
        Wx.reshape(L, 24, 128, 4, 128).transpose(0, 4, 1, 3, 2).reshape(L, 128, 12288)
    ).astype(bf16)
    wht = np.ascontiguousarray(
        Wh.reshape(L, 20, 128, 4, 128).transpose(0, 4, 1, 3, 2).reshape(L, 128, 10240)
    ).astype(bf16)
    # per-partition bias columns: bhb[l, p, m] = bh[l, m*128 + p], m in [0,20)
    bhb = np.zeros((L, 128, 24), dtype=np.float32)
    bhb[:, :, :20] = bh.reshape(L, 20, 128).transpose(0, 2, 1)

    in_maps = []
    for ci in range(NCORES):
        xs = x[ci * BL:(ci + 1) * BL]                      # [4, T, D]
        ls = lengths[ci * BL:(ci + 1) * BL]                # [4]
        # x_t[p, (t+1)*16 + kc*4 + b] = xs[b, t, kc*128+p]
        xt = np.zeros((128, T + 2, 4, 4), dtype=bf16)
        xt[:, 1:T + 1] = xs.reshape(BL, T, 4, 128).transpose(3, 1, 2, 0).astype(bf16)
        # pad-bias tensor mb[p, t*4+b] = 0 if t < len_b else -1e9
        m = (np.arange(T)[None, :] >= ls[:, None]).T       # [T, 4] padded=True
        mb = np.broadcast_to((m * -1e9).astype(bf16)[None, :, :], (128, T, 4))
        in_maps.append({
            "x_t": np.ascontiguousarray(xt.reshape(128, HS_W)),
            "mb": np.ascontiguousarray(mb.reshape(128, T * 4)),
            "ident": np.eye(128, dtype=bf16),
            "wxt": wxt,
            "wht": wht,
            "bhb": bhb,
        })
    return in_maps


def _patch_tile_wait_splitting():
    """This container's walrus rejects >1 sync wait per instruction
    ("Too many sync wait commands"). Split multi-wait instructions into
    single-wait nop carriers on the same engine."""
    import concourse.mybir as mybir
    import concourse.tile as tile_mod
    from concourse.tile import TileContext

    if getattr(TileContext, "_wait_split_patched", False):
        return
    TileContext._wait_split_patched = True

    _orig_add = TileContext._add_instruction

    def _split_add(self, inst):
        si = inst.sync_info
        if si is not None and si.on_wait and len(si.on_wait) > 1 \
           and inst.engine != mybir.EngineType.Unassigned:
            waits = list(si.on_wait)
            eng = self.nc.engines[inst.engine]
            for w in waits[:-1]:
                nop = eng.nop(nofuse=True)
                nop.ins.sync_info = mybir.SyncInfo(on_wait=[w], on_update=[])
                _orig_add(self, nop.ins)
            si.on_wait = [waits[-1]]
            inst.sync_info = si
        return _orig_add(self, inst)
    TileContext._add_instruction = _split_add

    def _patched_dab(self, tick_clock, wait_clock):
        ScopedClock = tile_mod.ScopedClock
        drain_inst = self.nc.sync.drain()
        wait_clock.add_sem_waits(
            drain_inst.ins, ScopedClock({None: tick_clock.global_clock}))
        si = drain_inst.ins.sync_info
        if si is not None and si.on_wait and len(si.on_wait) > 1:
            waits = list(si.on_wait)
            si.on_wait = [waits[0]]
            drain_inst.ins.sync_info = si
            for w in waits[1:]:
                nop = self.nc.sync.nop(nofuse=True)
                nop.ins.sync_info = mybir.SyncInfo(on_wait=[w], on_update=[])
        self.nc.all_engine_barrier()
        popped = self.nc._tile_sem_poison_stack.pop()
        assert popped is self._sem_poison
        self.nc.clear_and_free_semaphores(list(self.sems.allocated().values()))
        self.nc.all_engine_barrier()
    TileContext._drain_and_barrier = _patched_dab


def build_nc(t_eff):
    """t_eff: number of recurrence steps (== max length over batch)."""
    import concourse.bass as bass
    import concourse.mybir as mybir
    from concourse.tile import TileContext
    from concourse.bass import ds
    from concourse.alu_op_type import AluOpType as ALU
    _patch_tile_wait_splitting()

    f32 = mybir.dt.float32
    bf16 = mybir.dt.bfloat16
    AF = mybir.ActivationFunctionType

    nc = bass.Bass()
    x_ext = nc.declare_dram_parameter("x_t", [128, HS_W], bf16, isOutput=False)
    id_ext = nc.declare_dram_parameter("ident", [128, 128], bf16, isOutput=False)
    mb_ext = nc.declare_dram_parameter("mb", [128, T * 4], bf16, isOutput=False)
    wx_ext = nc.declare_dram_parameter("wxt", [L, 128, 12288], bf16, isOutput=False)
    wh_ext = nc.declare_dram_parameter("wht", [L, 128, 10240], bf16, isOutput=False)
    bh_ext = nc.declare_dram_parameter("bhb", [L, 128, 24], f32, isOutput=False)
    out_ext = nc.declare_dram_parameter("out", [128, T * 16], bf16, isOutput=True)

    n_tc = (t_eff + 127) // 128          # phase-A time chunks

    with TileContext(nc) as tc:
        with (
            tc.tile_pool(name="hs", bufs=2) as hspool,
            tc.tile_pool(name="xg", bufs=1) as xgpool,
            tc.tile_pool(name="wx", bufs=1) as wxpool,
            tc.tile_pool(name="wh", bufs=1) as whpool,
            tc.tile_pool(name="small", bufs=1) as smpool,
            tc.tile_pool(name="tmp", bufs=2) as tmp,
            tc.tile_pool(name="psA", bufs=2, space="PSUM") as psA,
            tc.tile_pool(name="psG", bufs=2, space="PSUM") as psG,
            tc.tile_pool(name="psW", bufs=1, space="PSUM") as psW,
        ):
            # scratch bank for PE keep-warm dummy matmuls (never read):
            # HAM throttles PE to 1.2 GHz unless it stays near-continuously
            # busy, so fill the post-step gap while DVE/ACT run the cell.
            warmps = psW.tile([128, 512], f32, tag="warm")
            xgbuf = xgpool.tile([128, XG_W], bf16, tag="xg")
            xg_r = xgbuf[:, :].rearrange("p (t x) -> p t x", x=96)
            # gate-indexed view [p, t, gate, hc, b]
            xg4 = xgbuf[:, :].rearrange(
                "p (t g hc b) -> p t g hc b", g=6, hc=4, b=4)

            mbuf = smpool.tile([128, T * 4], bf16, tag="mb")
            nc.sync.dma_start(out=mbuf[:, :], in_=mb_ext[:, :])
            # [p, t, 1, b] -> broadcast over hc
            mb_bc = mbuf[:, :].rearrange(
                "p (t one b) -> p t one b", one=1, b=4)[
                :, 0:t_eff].broadcast_to([128, t_eff, 4, 4])

            # persistent state tiles
            hfix0 = smpool.tile([128, 16], bf16, tag="hfix0")
            hfix1 = smpool.tile([128, 16], bf16, tag="hfix1")
            hfix = [hfix0, hfix1]
            # gc[:, 0:16] = tanh(g-gate), gc[:, 16:32] = c state
            gc = smpool.tile([128, 32], f32, tag="gc")
            ident = smpool.tile([128, 128], bf16, tag="ident")
            nc.sync.dma_start(out=ident[:, :], in_=id_ext[:, :])
            # per-chunk staging: xg window in, h slots out (all static APs
            # inside the step body; one dynamic AP per chunk per engine)
            xstage = smpool.tile([128, UNROLL * 96], bf16, tag="xstage")
            hstage = smpool.tile([128, UNROLL * 16], bf16, tag="hstage")

            hsprev = hspool.tile([128, HS_W], bf16, tag="hs")
            nc.sync.dma_start(out=hsprev[:, :], in_=x_ext[:, :])

            for l in range(NL):
                rev = (l % 2 == 1)
                # ---- Phase A: xg[t] = x_l[t] @ Wx_l.T + bh_l ----
                wtile = wxpool.tile([128, 12288], bf16, tag="wx")
                nc.sync.dma_start(out=wtile[:, :], in_=wx_ext[l, :, :])
                bcol = smpool.tile([128, 24], f32, tag="bcol")
                nc.sync.dma_start(out=bcol[:, :], in_=bh_ext[l, :, :])
                hs_r = hsprev[:, 16:16 + T * 16].rearrange(
                    "p (t x) -> p t x", x=16)
                for m in range(24):
                    gx, hc = m // 4, m % 4
                    for tcn in range(n_tc):
                        tw = min(128, t_eff - tcn * 128)
                        ps = psA.tile([128, 512], f32, tag="psA")
                        ps3 = ps[:, :tw * 4].rearrange("p (t b) -> p t b", b=4)
                        for k in range(4):
                            nc.tensor.matmul(
                                ps3,
                                lhsT=wtile[:, (m * 4 + k) * 128:(m * 4 + k + 1) * 128],
                                rhs=hs_r[:, tcn * 128:tcn * 128 + tw, k * 4:k * 4 + 4],
                                start=(k == 0), stop=(k == 3),
                            )
                        dst = xg_r[:, tcn * 128:tcn * 128 + tw,
                                   gx * 16 + hc * 4:gx * 16 + hc * 4 + 4]
                        if m % 2 == 0:
                            nc.scalar.activation(
                                dst, ps3, AF.Identity,
                                bias=bcol[:, m:m + 1], scale=1.0)
                        else:
                            nc.vector.tensor_scalar_add(
                                dst, ps3, bcol[:, m:m + 1])
                # padded positions: i,f get -1e9 (sigmoid -> 0), r gets +1e9
                # (sigmoid -> 1) so the cell emits exact zeros there.
                nc.gpsimd.tensor_add(
                    xg4[:, 0:t_eff, 0], xg4[:, 0:t_eff, 0], mb_bc)
                nc.gpsimd.tensor_add(
                    xg4[:, 0:t_eff, 1], xg4[:, 0:t_eff, 1], mb_bc)
                nc.gpsimd.tensor_sub(
                    xg4[:, 0:t_eff, 4], xg4[:, 0:t_eff, 4], mb_bc)

                # ---- Phase B: recurrence ----
                whtile = whpool.tile([128, 10240], bf16, tag="wh")
                nc.sync.dma_start(out=whtile[:, :], in_=wh_ext[l, :, :])
                hscur = hspool.tile([128, HS_W], bf16, tag="hs")
                # zero tail slots [t_eff+1, T+1] for output correctness
                if t_eff < T + 1:
                    nc.vector.memset(
                        hscur[:, (t_eff + 1) * 16:(T + 2) * 16], 0.0)
                nc.vector.memset(gc[:, 16:32], 0.0)
                nc.vector.memset(hfix[1][:, :], 0.0)

                def body(xsrc, xoff, hdst, par, whtile=whtile):
                    # xsrc[:, xoff:...] is a static AP (xstage slot or xgbuf
                    # tail offset); hdst is a static h output slice
                    rd = hfix[1 - par]
                    wr = hfix[par]
                    g1 = psG.tile([128, 512], f32, tag="g1")
                    g2 = psG.tile([128, 512], f32, tag="g2")
                    # xg injected FIRST with start=True: sets has_written for
                    # the whole used range, so every Wh matmul accumulates
                    # onto it (a later start=True would clear the bank's
                    # has_written bits and break accumulation).
                    nc.tensor.matmul(
                        g1[:, 0:48], lhsT=ident[:, :],
                        rhs=xsrc[:, xoff:xoff + 48],
                        start=True, stop=False, skip_group_check=True)
                    for m in range(12):          # i, f, g gates
                        for k in range(4):
                            nc.tensor.matmul(
                                g1[:, m * 4:m * 4 + 4],
                                lhsT=whtile[:, (m * 4 + k) * 128:(m * 4 + k + 1) * 128],
                                rhs=rd[:, k * 4:k * 4 + 4],
                                start=False, stop=(m == 11 and k == 3),
                                skip_group_check=True,
                            )
                    nc.tensor.matmul(
                        g2[:, 0:32], lhsT=ident[:, :],
                        rhs=xsrc[:, xoff + 48:xoff + 80],
                        start=True, stop=False, skip_group_check=True)
                    # r gates (cols 16:32) BEFORE o gates: sigmoid(r) and qn
                    # then overlap the o matmuls, leaving only the o tail
                    # exposed after the last matmul
                    for m in (16, 17, 18, 19, 12, 13, 14, 15):
                        for k in range(4):
                            nc.tensor.matmul(
                                g2[:, (m - 12) * 4:(m - 12) * 4 + 4],
                                lhsT=whtile[:, (m * 4 + k) * 128:(m * 4 + k + 1) * 128],
                                rhs=rd[:, k * 4:k * 4 + 4],
                                start=False, stop=(m == 15 and k == 3),
                                skip_group_check=True,
                            )
                    # i,f sigmoids; tanh(g) lands next to c in gc
                    s1 = tmp.tile([128, 32], f32, tag="s1")
                    nc.scalar.activation(s1[:, :], g1[:, 0:32], AF.Sigmoid)
                    nc.scalar.activation(gc[:, 0:16], g1[:, 32:48], AF.Tanh)
                    # [i|f] * [tanh(g)|c_prev] in one op, then c = halves sum
                    t12 = tmp.tile([128, 32], f32, tag="t12")
                    nc.vector.tensor_mul(t12[:, :], s1[:, :], gc[:, :])
                    nc.vector.tensor_add(gc[:, 16:32], t12[:, 0:16], t12[:, 16:32])
                    # sigmoid(r) as soon as the r matmuls land (o still going)
                    s2r = tmp.tile([128, 16], f32, tag="s2r")
                    nc.scalar.activation(s2r[:, :], g2[:, 16:32], AF.Sigmoid)
                    tct = tmp.tile([128, 16], f32, tag="tct")
                    nc.scalar.activation(tct[:, :], gc[:, 16:32], AF.Tanh)
                    qn = tmp.tile([128, 16], f32, tag="qn")
                    nc.vector.scalar_tensor_tensor(
                        qn[:, :], s2r[:, :], 1.0,
                        xsrc[:, xoff + 80:xoff + 96],
                        op0=ALU.subtract, op1=ALU.mult)
                    # rt = r * tanh(c) also lands before the o matmuls finish
                    rt = tmp.tile([128, 16], f32, tag="rt")
                    nc.vector.tensor_mul(rt[:, :], s2r[:, :], tct[:, :])
                    # exposed tail: sigmoid(o) -> h = o*rt - qn
                    s2o = tmp.tile([128, 16], f32, tag="s2o")
                    nc.scalar.activation(s2o[:, :], g2[:, 0:16], AF.Sigmoid)
                    u = tmp.tile([128, 16], f32, tag="u")
                    nc.vector.tensor_mul(u[:, :], s2o[:, :], rt[:, :])
                    nc.vector.tensor_sub(wr[:, :], u[:, :], qn[:, :])
                    nc.gpsimd.tensor_copy(hdst, wr[:, :])
                    for _ in range(NDUMMY):
                        nc.tensor.matmul(
                            warmps[:, :], lhsT=ident[:, :],
                            rhs=whtile[:, 0:512],
                            start=True, stop=True, skip_group_check=True)

                n_main = (t_eff // UNROLL) * UNROLL
                U = UNROLL
                if not rev:
                    if n_main > 0:
                        with tc.For_i(0, n_main, U, staggered_reset=STAGGER) as iv0:
                            # stage xg window for the chunk (t = iv0..iv0+U-1)
                            nc.vector.tensor_copy(
                                xstage[:, :], xgbuf[:, ds(iv0 * 96, U * 96)])
                            for u in range(U):
                                body(xstage, u * 96,
                                     hstage[:, u * 16:(u + 1) * 16], u % 2)
                            # flush h slots iv0+1 .. iv0+U
                            nc.gpsimd.tensor_copy(
                                hscur[:, ds(iv0 * 16 + 16, U * 16)],
                                hstage[:, :])
                    for j in range(n_main, t_eff):
                        body(xgbuf, j * 96,
                             hscur[:, (j + 1) * 16:(j + 2) * 16], j % 2)
                else:
                    if n_main > 0:
                        with tc.For_i(t_eff - 1, t_eff - 1 - n_main, -U, staggered_reset=STAGGER) as iv0:
                            # chunk covers t = iv0-U+1 .. iv0, processed
                            # descending; xstage slot s holds t = iv0-U+1+s
                            nc.vector.tensor_copy(
                                xstage[:, :],
                                xgbuf[:, ds(iv0 * 96 - (U - 1) * 96, U * 96)])
                            for u in range(U):
                                s = U - 1 - u
                                body(xstage, s * 96,
                                     hstage[:, s * 16:(s + 1) * 16], u % 2)
                            # flush h slots iv0-U+2 .. iv0+1
                            nc.gpsimd.tensor_copy(
                                hscur[:, ds(iv0 * 16 - (U - 2) * 16, U * 16)],
                                hstage[:, :])
                    for j in range(n_main, t_eff):
                        jv = t_eff - 1 - j
                        body(xgbuf, jv * 96,
                             hscur[:, (jv + 1) * 16:(jv + 2) * 16], j % 2)
                hsprev = hscur

            # ---- output: hsprev slots 1..T -> out [128, T*16] bf16 ----
            nc.sync.dma_start(out=out_ext[:, :], in_=hsprev[:, 16:16 + T * 16])
    return nc


_CACHED = {}


def kernel(x, lengths, Wx, Wh, bh):
    import sys
    for p in ("/opt/trn_rl_repo",):
        if p not in sys.path:
            sys.path.insert(0, p)
    from concourse.bass_utils import run_bass_kernel_spmd

    x = np.asarray(x, dtype=np.float32)
    lengths = np.asarray(lengths, dtype=np.int32)
    Wx = np.asarray(Wx, dtype=np.float32)
    Wh = np.asarray(Wh, dtype=np.float32)
    bh = np.asarray(bh, dtype=np.float32)

    t_eff = int(lengths.max())
    in_maps = _prep_host(x, lengths, Wx, Wh, bh)
    key = ("nc", t_eff)
    if key not in _CACHED:
        _CACHED[key] = build_nc(t_eff)
    nc = _CACHED[key]
    trace = bool(int(os.environ.get("KERNEL_TRACE", "0")))
    res = run_bass_kernel_spmd(nc, in_maps, core_ids=list(range(NCORES)),
                               trace=trace)
    _CACHED["exec_time_ns"] = getattr(res, "exec_time_ns", None)
    _CACHED["profile_json"] = getattr(res, "profile_json", None)
    _CACHED["res"] = res

    bf = np.stack([np.asarray(r["out"]) for r in res.results])  # [8,128,T*16]
    # out[ci, p, t*16 + hc*4 + b] -> full [B, T, H] f32
    hb = bf.reshape(NCORES, 128, T, 4, 4).transpose(0, 4, 2, 3, 1)
    full = hb.reshape(B, T, H).astype(np.float32)
    return full


if __name__ == "__main__":
    nc = build_nc(497)
    print("build ok")

